# revision 1
# baseline (speedup 1.0000x reference)
"""Trainium2 Bass kernel for nn_Cifar10_JointMembership.

Math (closed form of the reference 2-qubit circuit; verified vs reference):
  a = x[b, i0], b_ = x[b, i1]  (gathered pixel pairs, full angles)
  out[b, 2p,   c] = 0.5 + 0.5*cos(theta_c)*cos(a) - 0.5*sin(theta_c)*sin(a)*sin(b_)
  out[b, 2p+1, c] = 0.5 + 0.5*cos(a)*cos(b_)               (same for all c)

Sharding: pure data parallel, batch dim split across 8 NeuronCores
(128 rows per core); theta replicated. Full inputs in, full output out.

Per-core pipeline:
  DMA x [128,3072] f32 and pair_idx [128,920] i32 -> SBUF
  u16 index extraction (bitcast + stride-2 copy)
  GPSIMD indirect_copy gather: each Q7 core's 16-partition group uses its
    own wrapped index list; output column i = 16*m + w holds row-w-of-group's
    m-th gathered value looked up in every partition of the group, so the
    valid value for partition p sits at column 16*m + (p % 16).
  Phase merge on the (otherwise idle) TensorEngine: 16 accumulated matmuls
    with diagonal 0/1 weights W_w = diag(p%16 == w) select the valid phase
    per partition into PSUM: packed[p,m] = sum_w W_w[p,p] * tmp[p, 16m+w].
    Exact in fp32 (one nonzero term per output).
  Range reduction with compare-wraps (is_gt/is_lt + mul-add) into
    [-pi, pi]; ACT Sin for sin, Sin(-|y|+pi/2) for cos; DVE products;
    per-class affine (ACT Copy with per-partition scale + DVE
    scalar_tensor_tensor); broadcast odd columns; DMA out.
"""

import os

os.environ.setdefault("BY_DEFAULT_DISABLE_SUBTILE_DEPS", "1")

import numpy as np

import concourse.bass as bass
import concourse.mybir as mybir
from concourse.tile import TileContext as _TileContext, ScopedClock

N_CORES = 8
B_FULL = 1024
B = B_FULL // N_CORES  # 128 rows per core
NPIX = 3072
NPAIR = 460
NIDX = 2 * NPAIR  # 920 gathered values per row
NCLS = 10
F32 = mybir.dt.float32
I32 = mybir.dt.int32
U16 = mybir.dt.uint16
ALU = mybir.AluOpType
PI = float(np.pi)
TWO_PI = float(2 * np.pi)
HALF_PI = float(np.pi / 2)


class TileContext(_TileContext):
    pass


def _legalize_sync_waits(nc):
    """This walrus build allows only ONE sync-wait per non-EventSemaphore
    instruction (and two on EventSemaphore). Tile's add_semaphores can attach
    several. Hoist excess waits onto EventSemaphore instructions inserted
    immediately before the owner on the same engine — semantically identical
    (same engine stream, waits run first)."""
    n_new = 0
    for f in nc.m.functions:
        for bb in f.blocks:
            out = []
            for inst in bb.instructions:
                si = inst.sync_info
                waits = list(si.on_wait) if si is not None and si.on_wait else []
                cap = 2 if inst.opcode == "EventSemaphore" else 1
                if len(waits) > cap:
                    keep, hoist = waits[:cap], waits[cap:]
                    del si.on_wait[:]
                    for w in keep:
                        si.on_wait.append(w)
                    while hoist:
                        chunk, hoist = hoist[:2], hoist[2:]
                        n_new += 1
                        ev = mybir.InstEventSemaphore(
                            name=f"{inst.name}-hw{n_new}",
                            ins=[],
                            outs=[],
                            engine=inst.engine,
                            sync_info=mybir.SyncInfo(on_wait=chunk, on_update=[]),
                        )
                        out.append(ev)
                out.append(inst)
            bb.instructions = out
    return nc


def build_kernel(n_chunks=4, n_repeat=1, pe_phases=12, span_chunks=(2, 1, 1), parts="gmtc"):
    """One NeuronCore's program: 128 batch rows.

    n_chunks: gather/merge/trig pipeline granularity (divides 920, even CH).
    pe_phases: how many of the 16 phase-merge terms run on the TensorEngine
      (accumulated in PSUM); the rest run as a masked mul-add chain on DVE.
    span_chunks: class/output-stage spans, in units of chunks (sums to
      n_chunks). A small final span shortens the non-overlapped tail.
    n_repeat: re-runs the whole pipeline (identical results) for timing.
    """
    Sin = mybir.ActivationFunctionType.Sin
    Copy = mybir.ActivationFunctionType.Copy
    Abs = mybir.ActivationFunctionType.Abs

    nc = bass.Bass(detect_race_conditions=False)
    xd = nc.dram_tensor("x", [B, NPIX], F32, kind="ExternalInput")
    pd = nc.dram_tensor("pidx", [B, NIDX], I32, kind="ExternalInput")
    td = nc.dram_tensor("theta", [1, NCLS], F32, kind="ExternalInput")
    od = nc.dram_tensor("out", [B, NIDX * NCLS], F32, kind="ExternalOutput")

    assert NIDX % n_chunks == 0
    CH = NIDX // n_chunks  # gathered values per chunk
    assert CH % 2 == 0
    PCH = CH // 2  # pairs per chunk
    assert sum(span_chunks) == n_chunks

    with TileContext(nc) as tc:
        with (
            tc.tile_pool(name="const", bufs=1) as cpool,
            tc.tile_pool(name="inp", bufs=1) as ipool,
            tc.tile_pool(name="tmp", bufs=3) as tpool,
            tc.tile_pool(name="mid", bufs=2) as mpool,
            tc.tile_pool(name="trig", bufs=1) as gpool,
            tc.tile_pool(name="outp", bufs=2) as opool,
            tc.tile_pool(name="tccp", bufs=4) as tccpool,
            tc.tile_pool(name="ps", bufs=2, space="PSUM") as ppool,
        ):
            # --- phase masks M[p, w] = 1.0 if p % 16 == w else 0.0 ---
            rowx = cpool.tile([B, 1], I32, tag="rowx")
            nc.gpsimd.iota(rowx[:], pattern=[[0, 1]], base=0, channel_multiplier=1)
            pm16 = cpool.tile([B, 1], I32, tag="pm16")
            nc.vector.tensor_scalar(pm16[:], rowx[:], 15, None, ALU.bitwise_and)
            wrow = cpool.tile([B, 16], I32, tag="wrow")
            nc.gpsimd.iota(wrow[:], pattern=[[1, 16]], base=0, channel_multiplier=0)
            M = cpool.tile([B, 16], F32, tag="M")
            nc.vector.tensor_tensor(
                M[:], pm16[:, 0:1].broadcast_to((B, 16)), wrow[:], ALU.is_equal
            )

            # --- coefficients: A = 0.5*cos(theta), Bc = -0.5*sin(theta) ---
            # (theta + indices ride the ACT HWDGE ring so the big x DMA on
            # the SP ring doesn't serialize ahead of them)
            th = cpool.tile([B, NCLS], F32, tag="th")
            nc.scalar.dma_start(out=th[:], in_=td[:].to_broadcast((B, NCLS)))
            halfpi = cpool.tile([B, 1], F32, tag="halfpi")
            nc.gpsimd.memset(halfpi[:], HALF_PI)
            zbias = cpool.tile([B, 1], F32, tag="zbias")
            nc.gpsimd.memset(zbias[:], 0.0)

            # Range reduction with standard ALUs (valid for |x| < 3pi):
            #   y = x - 2pi*(x > pi) + 2pi*(x < -pi)  in [-pi, pi]
            #   sin(x) = Sin(y);  cos(x) = cos(|y|) = Sin(-|y| + pi/2)
            def wrap2(dst_y, src, g, l, y1, cmp_engine):
                cmp_engine.tensor_scalar(g, src, PI, None, ALU.is_gt)
                cmp_engine.tensor_scalar(l, src, -PI, None, ALU.is_lt)
                nc.vector.scalar_tensor_tensor(
                    y1, g, -TWO_PI, src, ALU.mult, ALU.add
                )
                nc.vector.scalar_tensor_tensor(
                    dst_y, l, TWO_PI, y1, ALU.mult, ALU.add
                )

            thy = cpool.tile([B, NCLS], F32, tag="thy")
            thz = cpool.tile([B, NCLS], F32, tag="thz")
            tg = cpool.tile([B, NCLS], F32, tag="tg")
            tl = cpool.tile([B, NCLS], F32, tag="tl")
            t1 = cpool.tile([B, NCLS], F32, tag="t1")
            wrap2(thy[:], th[:], tg[:], tl[:], t1[:], nc.vector)
            nc.scalar.activation(thz[:], thy[:], Abs, bias=zbias[:, 0:1])
            A = cpool.tile([B, NCLS], F32, tag="A")
            Bc = cpool.tile([B, NCLS], F32, tag="Bc")
            nc.scalar.activation(A[:], thz[:], Sin, bias=halfpi[:, 0:1], scale=-1.0)
            nc.scalar.activation(Bc[:], thy[:], Sin, bias=zbias[:, 0:1])
            nc.vector.tensor_scalar_mul(A[:], A[:], 0.5)
            nc.vector.tensor_scalar_mul(Bc[:], Bc[:], -0.5)

            if pe_phases > 0:
                # --- diagonal selection weights W_w = diag(M[:, w]) ---
                colx = cpool.tile([B, 128], I32, tag="colx")
                nc.gpsimd.iota(colx[:], pattern=[[1, 128]], base=0, channel_multiplier=0)
                D = cpool.tile([B, 128], F32, tag="D")
                nc.vector.tensor_tensor(
                    D[:], rowx[:, 0:1].broadcast_to((B, 128)), colx[:], ALU.is_equal
                )
                W_all = cpool.tile([B, pe_phases * 128], F32, tag="W_all")
                for w in range(pe_phases):
                    nc.vector.tensor_scalar(
                        W_all[:, w * 128 : (w + 1) * 128],
                        D[:],
                        M[:, w : w + 1],
                        None,
                        ALU.mult,
                    )

            # --- inputs ---
            i32 = ipool.tile([B, NIDX], I32, tag="i32")
            nc.scalar.dma_start(out=i32[:], in_=pd[:])
            xt = ipool.tile([B, NPIX], F32, tag="xt")
            nc.sync.dma_start(out=xt[:], in_=xd[:])
            i16 = ipool.tile([B, NIDX], U16, tag="i16")
            nc.vector.tensor_copy(i16[:], i32[:].bitcast(U16)[:, 0 : 2 * NIDX : 2])

            if pe_phases > 0:
                # warm the PE (HAM) during the prologue so the first real
                # merge matmuls run at full rate
                pwarm = ppool.tile([B, 64], F32, tag="pwarm")
                for _ in range(6):
                    nc.tensor.matmul(
                        pwarm[:], W_all[:, 0:128], D[:, 0:64], start=True, stop=True
                    )

            def class_span(p0, PS, ca, sa, cb, sb):
                """Per-class output stage for pairs [p0, p0+PS)."""
                sl = slice(p0, p0 + PS)
                v = mpool.tile([B, PS], F32, tag="v")
                wv = mpool.tile([B, PS], F32, tag="wv")
                nc.vector.tensor_mul(v[:], sa[:, sl], sb[:, sl])
                nc.vector.tensor_mul(wv[:], ca[:, sl], cb[:, sl])

                ob = opool.tile([B, PS * 2 * NCLS], F32, tag="ob")
                ob3 = ob[:].rearrange("p (t k) -> p t k", k=2 * NCLS)

                om = mpool.tile([B, PS], F32, tag="om")
                nc.scalar.activation(om[:], wv[:], Copy, bias=0.5, scale=0.5)
                nc.scalar.activation(
                    ob3[:, :, NCLS : 2 * NCLS],
                    om[:, :, None].broadcast_to((B, PS, NCLS)),
                    Copy,
                )

                # even rows per class c: A_c*ca + (Bc_c*v + 0.5)
                for c in range(NCLS):
                    tcc = tccpool.tile([B, PS], F32, tag="tcc")
                    nc.scalar.activation(
                        tcc[:], v[:], Copy, bias=0.5, scale=Bc[:, c : c + 1]
                    )
                    nc.vector.scalar_tensor_tensor(
                        ob[:, c : PS * 2 * NCLS : 2 * NCLS],
                        ca[:, sl],
                        A[:, c : c + 1],
                        tcc[:],
                        ALU.mult,
                        ALU.add,
                    )

                nc.sync.dma_start(
                    out=od[:, p0 * 2 * NCLS : (p0 + PS) * 2 * NCLS],
                    in_=ob[:],
                )

            for rep in range(n_repeat):
                # full-width cos/sin buffers, filled per chunk
                ca = gpool.tile([B, NPAIR], F32, tag="ca")
                sa = gpool.tile([B, NPAIR], F32, tag="sa")
                cb = gpool.tile([B, NPAIR], F32, tag="cb")
                sb = gpool.tile([B, NPAIR], F32, tag="sb")

                for k in range(n_chunks):
                    # --- gather (this walrus caps indirect_copy at 64
                    # index-columns per instruction, so sub-gather) ---
                    tmp = tpool.tile([B, CH * 16], F32, tag="tmp")
                    GSUB = 64
                    for g0 in [] if "g" not in parts else range(0, CH, GSUB):
                        gn = min(GSUB, CH - g0)
                        nc.gpsimd.indirect_copy(
                            tmp[:, 16 * g0 : 16 * (g0 + gn)],
                            xt[:],
                            i16[:, k * CH + g0 : k * CH + g0 + gn],
                            True,
                        )

                    # --- phase merge -> pk[p, m] (interleaved a,b) ---
                    if "m" not in parts:
                        pk = mpool.tile([B, CH], F32, tag="acc")
                    elif pe_phases > 0:
                        pk = ppool.tile([B, CH], F32, tag="pk")
                        for w in range(pe_phases):
                            nc.tensor.matmul(
                                pk[:],
                                W_all[:, w * 128 : (w + 1) * 128],
                                tmp[:, w : CH * 16 : 16],
                                start=(w == 0),
                                stop=(w == pe_phases - 1),
                            )
                    if "m" in parts and pe_phases < 16:
                        w0 = pe_phases
                        acc = mpool.tile([B, CH], F32, tag="acc")
                        nc.vector.tensor_scalar(
                            acc[:], tmp[:, w0 : CH * 16 : 16], M[:, w0 : w0 + 1],
                            None, ALU.mult,
                        )
                        for w in range(w0 + 1, 16):
                            nc.vector.scalar_tensor_tensor(
                                acc[:],
                                tmp[:, w : CH * 16 : 16],
                                M[:, w : w + 1],
                                acc[:],
                                ALU.mult,
                                ALU.add,
                            )
                        if pe_phases > 0:
                            nc.vector.tensor_add(acc[:], acc[:], pk[:])
                        pk = acc

                    # --- range reduction + trig -> full-width buffers ---
                    # (pk is SBUF when the DVE-assist merge ran; evacuate
                    # PSUM via ACT first when the full merge was on PE, so
                    # the GPSIMD compares can read it)
                    if pe_phases == 16:
                        pksb = mpool.tile([B, CH], F32, tag="pksb")
                        nc.scalar.activation(pksb[:], pk[:], Copy)
                        pk = pksb
                    av = pk[:, 0:CH:2]
                    bv = pk[:, 1:CH:2]
                    aw = mpool.tile([B, PCH], F32, tag="aw")
                    ac = mpool.tile([B, PCH], F32, tag="ac")
                    bw = mpool.tile([B, PCH], F32, tag="bw")
                    bc2 = mpool.tile([B, PCH], F32, tag="bc2")
                    ga = mpool.tile([B, PCH], F32, tag="ga")
                    la = mpool.tile([B, PCH], F32, tag="la")
                    gb = mpool.tile([B, PCH], F32, tag="gb")
                    lb = mpool.tile([B, PCH], F32, tag="lb")
                    if "t" not in parts:
                        continue
                    wrap2(aw[:], av, ga[:], la[:], ac[:], nc.vector)
                    nc.scalar.activation(ac[:], aw[:], Abs, bias=zbias[:, 0:1])
                    wrap2(bw[:], bv, gb[:], lb[:], bc2[:], nc.vector)
                    nc.scalar.activation(bc2[:], bw[:], Abs, bias=zbias[:, 0:1])

                    sl = slice(k * PCH, (k + 1) * PCH)
                    nc.scalar.activation(
                        ca[:, sl], ac[:], Sin, bias=halfpi[:, 0:1], scale=-1.0
                    )
                    nc.scalar.activation(sa[:, sl], aw[:], Sin, bias=zbias[:, 0:1])
                    nc.scalar.activation(
                        cb[:, sl], bc2[:], Sin, bias=halfpi[:, 0:1], scale=-1.0
                    )
                    nc.scalar.activation(sb[:, sl], bw[:], Sin, bias=zbias[:, 0:1])

                    # interleave the class/output stage as soon as its
                    # span of pairs is complete
                    done = k + 1
                    acc_ch = 0
                    for nch in [] if "c" not in parts else span_chunks:
                        if acc_ch + nch == done:
                            class_span(acc_ch * PCH, nch * PCH, ca, sa, cb, sb)
                            break
                        acc_ch += nch
    return _legalize_sync_waits(nc)


def _prep_inputs(x, theta, pair_idx):
    """Full inputs -> list of per-core input maps (host-side sharding only)."""
    x = np.ascontiguousarray(np.asarray(x, dtype=np.float32).reshape(B_FULL, NPIX))
    theta = np.ascontiguousarray(np.asarray(theta, dtype=np.float32).reshape(1, NCLS))
    pidx = np.asarray(pair_idx)
    assert pidx.shape == (B_FULL, NPAIR, 2), pidx.shape
    if pidx.dtype != np.int32:
        pidx = pidx.astype(np.int32)  # value-preserving narrowing for the DMA
    pidx = np.ascontiguousarray(pidx.reshape(B_FULL, NIDX))
    in_maps = []
    for k in range(N_CORES):
        sl = slice(k * B, (k + 1) * B)
        in_maps.append({"x": x[sl], "pidx": pidx[sl], "theta": theta})
    return in_maps


_CACHED = {}


def kernel(x, theta, pair_idx):
    from concourse.bass_utils import run_bass_kernel_spmd

    if "nc" not in _CACHED:
        _CACHED["nc"] = build_kernel()
    nc = _CACHED["nc"]
    in_maps = _prep_inputs(x, theta, pair_idx)
    res = run_bass_kernel_spmd(nc, in_maps, core_ids=list(range(N_CORES)))
    out = np.concatenate([r["out"] for r in res.results], axis=0)
    return out.reshape(B_FULL, NIDX, NCLS)



# revision 7
# speedup vs baseline: 12.6460x; 12.6460x over previous
"""Trainium2 Bass kernel for nn_Cifar10_JointMembership.

Math (closed form of the reference 2-qubit circuit; verified vs reference):
  a = x[b, i0], b_ = x[b, i1]  (gathered pixel pairs, full angles)
  out[b, 2p,   c] = 0.5 + 0.5*cos(theta_c)*cos(a) - 0.5*sin(theta_c)*sin(a)*sin(b_)
  out[b, 2p+1, c] = 0.5 + 0.5*cos(a)*cos(b_)               (same for all c)

Sharding: pure data parallel, batch dim split across 8 NeuronCores
(128 rows per core); theta replicated. Full inputs in, full output out.

Gather strategy: the only per-partition-independent indexed primitive on
TRN2's GPSIMD is `local_scatter` (per-lane scatter through Q7-local RAM at
streaming rate; the SBUF-read gathers `indirect_copy`/`ap_gather` pay a
non-pipelined ~30 cyc per gathered column). A gather is the inverse of a
scatter, so the host re-encodes pair_idx (pure index bookkeeping, no data
touched) into scatter form:
  inv[b, pix] = first output slot j with pair_idx[b, j] == pix  (or -1)
  C1[b, j0]   = second-occurrence slot fed from first-occurrence slot j0
  Pc[b, j0]   = compaction slot (0..63) for pixels used >= 3 times
  Cc[b, (k-2)*64 + slot] = slot j of the k-th occurrence, k >= 2
Device pipeline per core:
  DMA x with SWDGE f32->fp16 cast; scatter xh by inv -> d0 (first
  occurrences); scatter d0 by C1 -> d1; scatter d0 by Pc -> comp (64
  slots); replicate comp, scatter by Cc -> dc; acc = d0+d1+dc (disjoint
  supports, exact). Then per chunk: fp16->f32, range reduction with
  compare-wraps into [-pi, pi], ACT Sin for sin and Sin(-|y|+pi/2) for
  cos, DVE products, per-class affine (ACT Copy with per-partition scale
  + DVE scalar_tensor_tensor), broadcast odd columns; DMA out.
fp16 gather values bound the error at ~1 ulp(5.5) ~ 2e-3 abs, well inside
the 2e-2 gate.
"""

import os

os.environ.setdefault("BY_DEFAULT_DISABLE_SUBTILE_DEPS", "1")

import numpy as np

import concourse.bass as bass
import concourse.mybir as mybir
from concourse import library_config
from concourse.tile import TileContext as _TileContext

N_CORES = 8
B_FULL = 1024
B = B_FULL // N_CORES  # 128 rows per core
NPIX = 3072
NPAIR = 460
NIDX = 2 * NPAIR  # 920 gathered values per row
NCLS = 10
NCOMP = 64  # compaction slots per row for pixels used >= 3 times
F32 = mybir.dt.float32
F16 = mybir.dt.float16
I16 = mybir.dt.int16
ALU = mybir.AluOpType
PI = float(np.pi)
TWO_PI = float(2 * np.pi)
HALF_PI = float(np.pi / 2)


class TileContext(_TileContext):
    pass


def _legalize_sync_waits(nc):
    """This walrus build allows only ONE sync-wait per non-EventSemaphore
    instruction (and two on EventSemaphore). Tile's add_semaphores can attach
    several. Hoist excess waits onto EventSemaphore instructions inserted
    immediately before the owner on the same engine — semantically identical
    (same engine stream, waits run first)."""
    n_new = 0
    for f in nc.m.functions:
        for bb in f.blocks:
            out = []
            for inst in bb.instructions:
                si = inst.sync_info
                waits = list(si.on_wait) if si is not None and si.on_wait else []
                cap = 2 if inst.opcode == "EventSemaphore" else 1
                if len(waits) > cap:
                    keep, hoist = waits[:cap], waits[cap:]
                    del si.on_wait[:]
                    for w in keep:
                        si.on_wait.append(w)
                    while hoist:
                        chunk, hoist = hoist[:2], hoist[2:]
                        n_new += 1
                        ev = mybir.InstEventSemaphore(
                            name=f"{inst.name}-hw{n_new}",
                            ins=[],
                            outs=[],
                            engine=inst.engine,
                            sync_info=mybir.SyncInfo(on_wait=chunk, on_update=[]),
                        )
                        out.append(ev)
                out.append(inst)
            bb.instructions = out
    return nc


def build_kernel(n_repeat=1, n_ranks=6, n_chunks=4, span_chunks=(2, 1, 1)):
    """One NeuronCore's program: 128 batch rows.

    n_ranks: max index multiplicity covered (rank 0 = first occurrence).
    n_chunks: trig/class pipeline granularity (divides 920, even CH).
    span_chunks: class/output-stage spans, in units of chunks (sums to
      n_chunks). A small final span shortens the non-overlapped tail.
    n_repeat: re-runs the whole pipeline (identical results) for timing.
    """
    Sin = mybir.ActivationFunctionType.Sin
    Copy = mybir.ActivationFunctionType.Copy
    Abs = mybir.ActivationFunctionType.Abs

    nc = bass.Bass(detect_race_conditions=False)
    xd = nc.dram_tensor("x", [B, NPIX], F32, kind="ExternalInput")
    invd = nc.dram_tensor("inv", [B, NPIX], I16, kind="ExternalInput")
    c1d = nc.dram_tensor("c1", [B, NIDX], I16, kind="ExternalInput")
    pcd = nc.dram_tensor("pc", [B, NIDX], I16, kind="ExternalInput")
    NCC = max(n_ranks - 2, 1) * NCOMP
    ccd = nc.dram_tensor("cc", [B, NCC], I16, kind="ExternalInput")
    td = nc.dram_tensor("theta", [1, NCLS], F32, kind="ExternalInput")
    od = nc.dram_tensor("out", [B, NIDX * NCLS], F32, kind="ExternalOutput")

    assert NIDX % n_chunks == 0
    CH = NIDX // n_chunks  # gathered values per chunk
    assert CH % 2 == 0
    PCH = CH // 2  # pairs per chunk
    assert sum(span_chunks) == n_chunks

    with TileContext(nc) as tc:
        with (
            tc.tile_pool(name="const", bufs=1) as cpool,
            tc.tile_pool(name="inp", bufs=1) as ipool,
            tc.tile_pool(name="gat", bufs=2) as gpool,
            tc.tile_pool(name="mid", bufs=2) as mpool,
            tc.tile_pool(name="trig", bufs=1) as tgpool,
            tc.tile_pool(name="outp", bufs=2) as opool,
            tc.tile_pool(name="tccp", bufs=4) as tccpool,
        ):
            # GPSIMD library for local_scatter; must precede every
            # library-tracked pool instruction (memset is built-in).
            nc.gpsimd.load_library(library_config.local_scatter)

            halfpi = cpool.tile([B, 1], F32, tag="halfpi")
            nc.gpsimd.memset(halfpi[:], HALF_PI)
            zbias = cpool.tile([B, 1], F32, tag="zbias")
            nc.gpsimd.memset(zbias[:], 0.0)

            # --- coefficients: A = 0.5*cos(theta), Bc = -0.5*sin(theta) ---
            th = cpool.tile([B, NCLS], F32, tag="th")
            nc.scalar.dma_start(out=th[:], in_=td[:].to_broadcast((B, NCLS)))

            # Range reduction with standard ALUs (valid for |x| < 3pi):
            #   y = x - 2pi*(x > pi) + 2pi*(x < -pi)  in [-pi, pi]
            #   sin(x) = Sin(y);  cos(x) = cos(|y|) = Sin(-|y| + pi/2)
            def wrap2(dst_y, src, g, l, y1):
                nc.vector.tensor_scalar(g, src, PI, None, ALU.is_gt)
                nc.vector.tensor_scalar(l, src, -PI, None, ALU.is_lt)
                nc.vector.scalar_tensor_tensor(y1, g, -TWO_PI, src, ALU.mult, ALU.add)
                nc.vector.scalar_tensor_tensor(dst_y, l, TWO_PI, y1, ALU.mult, ALU.add)

            thy = cpool.tile([B, NCLS], F32, tag="thy")
            thz = cpool.tile([B, NCLS], F32, tag="thz")
            tg = cpool.tile([B, NCLS], F32, tag="tg")
            tl = cpool.tile([B, NCLS], F32, tag="tl")
            t1 = cpool.tile([B, NCLS], F32, tag="t1")
            wrap2(thy[:], th[:], tg[:], tl[:], t1[:])
            nc.scalar.activation(thz[:], thy[:], Abs, bias=zbias[:, 0:1])
            A = cpool.tile([B, NCLS], F32, tag="A")
            Bc = cpool.tile([B, NCLS], F32, tag="Bc")
            nc.scalar.activation(A[:], thz[:], Sin, bias=halfpi[:, 0:1], scale=-1.0)
            nc.scalar.activation(Bc[:], thy[:], Sin, bias=zbias[:, 0:1])
            nc.vector.tensor_scalar_mul(A[:], A[:], 0.5)
            nc.vector.tensor_scalar_mul(Bc[:], Bc[:], -0.5)

            # --- inputs ---
            # x cast to fp16 during the DMA itself (SWDGE converts in the
            # SDMA datapath; no engine time).
            xh = ipool.tile([B, NPIX], F16, tag="xh")
            nc.gpsimd.dma_start(out=xh[:], in_=xd[:])
            invt = ipool.tile([B, NPIX], I16, tag="invt")
            nc.scalar.dma_start(out=invt[:], in_=invd[:])
            c1t = ipool.tile([B, NIDX], I16, tag="c1t")
            nc.scalar.dma_start(out=c1t[:], in_=c1d[:])
            pct = ipool.tile([B, NIDX], I16, tag="pct")
            nc.scalar.dma_start(out=pct[:], in_=pcd[:])
            cct = ipool.tile([B, NCC], I16, tag="cct")
            nc.scalar.dma_start(out=cct[:], in_=ccd[:])

            def class_span(p0, PS, ca, sa, cb, sb):
                """Per-class output stage for pairs [p0, p0+PS)."""
                sl = slice(p0, p0 + PS)
                v = mpool.tile([B, PS], F32, tag="v")
                wv = mpool.tile([B, PS], F32, tag="wv")
                nc.vector.tensor_mul(v[:], sa[:, sl], sb[:, sl])
                nc.vector.tensor_mul(wv[:], ca[:, sl], cb[:, sl])

                ob = opool.tile([B, PS * 2 * NCLS], F32, tag="ob")
                ob3 = ob[:].rearrange("p (t k) -> p t k", k=2 * NCLS)

                om = mpool.tile([B, PS], F32, tag="om")
                nc.scalar.activation(om[:], wv[:], Copy, bias=0.5, scale=0.5)
                nc.scalar.activation(
                    ob3[:, :, NCLS : 2 * NCLS],
                    om[:, :, None].broadcast_to((B, PS, NCLS)),
                    Copy,
                )

                # even rows per class c: A_c*ca + (Bc_c*v + 0.5)
                for c in range(NCLS):
                    tcc = tccpool.tile([B, PS], F32, tag="tcc")
                    nc.scalar.activation(
                        tcc[:], v[:], Copy, bias=0.5, scale=Bc[:, c : c + 1]
                    )
                    nc.vector.scalar_tensor_tensor(
                        ob[:, c : PS * 2 * NCLS : 2 * NCLS],
                        ca[:, sl],
                        A[:, c : c + 1],
                        tcc[:],
                        ALU.mult,
                        ALU.add,
                    )

                nc.sync.dma_start(
                    out=od[:, p0 * 2 * NCLS : (p0 + PS) * 2 * NCLS],
                    in_=ob[:],
                )

            for rep in range(n_repeat):
                # --- scatter-gather: acc[b, j] = x[b, pair_idx[b, j]] ---
                d0 = gpool.tile([B, NIDX], F16, tag="d0")
                nc.gpsimd.local_scatter(
                    d0[:], xh[:], invt[:], channels=B, num_elems=NIDX, num_idxs=NPIX
                )
                d1 = gpool.tile([B, NIDX], F16, tag="d1")
                nc.gpsimd.local_scatter(
                    d1[:], d0[:], c1t[:], channels=B, num_elems=NIDX, num_idxs=NIDX
                )
                acc = gpool.tile([B, NIDX], F16, tag="acch")
                if n_ranks > 2:
                    comp = gpool.tile([B, NCOMP], F16, tag="comp")
                    nc.gpsimd.local_scatter(
                        comp[:], d0[:], pct[:],
                        channels=B, num_elems=NCOMP, num_idxs=NIDX,
                    )
                    compr = gpool.tile([B, NCC], F16, tag="compr")
                    for t in range(n_ranks - 2):
                        nc.vector.tensor_copy(
                            compr[:, t * NCOMP : (t + 1) * NCOMP], comp[:]
                        )
                    dc = gpool.tile([B, NIDX], F16, tag="dc")
                    nc.gpsimd.local_scatter(
                        dc[:], compr[:], cct[:],
                        channels=B, num_elems=NIDX, num_idxs=NCC,
                    )
                    nc.vector.tensor_add(acc[:], d0[:], d1[:])
                    nc.vector.tensor_add(acc[:], acc[:], dc[:])
                else:
                    nc.vector.tensor_add(acc[:], d0[:], d1[:])

                # full-width cos/sin buffers, filled per chunk
                ca = tgpool.tile([B, NPAIR], F32, tag="ca")
                sa = tgpool.tile([B, NPAIR], F32, tag="sa")
                cb = tgpool.tile([B, NPAIR], F32, tag="cb")
                sb = tgpool.tile([B, NPAIR], F32, tag="sb")

                for k in range(n_chunks):
                    pk = mpool.tile([B, CH], F32, tag="pk")
                    nc.scalar.activation(
                        pk[:], acc[:, k * CH : (k + 1) * CH], Copy
                    )
                    av = pk[:, 0:CH:2]
                    bv = pk[:, 1:CH:2]
                    aw = mpool.tile([B, PCH], F32, tag="aw")
                    ac = mpool.tile([B, PCH], F32, tag="ac")
                    bw = mpool.tile([B, PCH], F32, tag="bw")
                    bc2 = mpool.tile([B, PCH], F32, tag="bc2")
                    ga = mpool.tile([B, PCH], F32, tag="ga")
                    la = mpool.tile([B, PCH], F32, tag="la")
                    gb = mpool.tile([B, PCH], F32, tag="gb")
                    lb = mpool.tile([B, PCH], F32, tag="lb")
                    wrap2(aw[:], av, ga[:], la[:], ac[:])
                    nc.scalar.activation(ac[:], aw[:], Abs, bias=zbias[:, 0:1])
                    wrap2(bw[:], bv, gb[:], lb[:], bc2[:])
                    nc.scalar.activation(bc2[:], bw[:], Abs, bias=zbias[:, 0:1])

                    sl = slice(k * PCH, (k + 1) * PCH)
                    nc.scalar.activation(
                        ca[:, sl], ac[:], Sin, bias=halfpi[:, 0:1], scale=-1.0
                    )
                    nc.scalar.activation(sa[:, sl], aw[:], Sin, bias=zbias[:, 0:1])
                    nc.scalar.activation(
                        cb[:, sl], bc2[:], Sin, bias=halfpi[:, 0:1], scale=-1.0
                    )
                    nc.scalar.activation(sb[:, sl], bw[:], Sin, bias=zbias[:, 0:1])

                    # interleave the class/output stage as soon as its
                    # span of pairs is complete
                    done = k + 1
                    acc_ch = 0
                    for nch in span_chunks:
                        if acc_ch + nch == done:
                            class_span(acc_ch * PCH, nch * PCH, ca, sa, cb, sb)
                            break
                        acc_ch += nch
    _legalize_sync_waits(nc)
    mybir.codegen_inst_isa_subclasses(nc)
    return nc


def _index_prep(pidx):
    """pair_idx [B_FULL, NIDX] (int, < NPIX) -> inv, c1, pc, cc, n_ranks.

    Pure index re-encoding (host touches no model data): the gather
    vals[b, j] = x[b, pidx[b, j]] becomes device scatters
      d0[inv[b,pix]] = x[b,pix]; d1[c1[b,j0]] = d0[j0];
      comp[pc[b,j0]] = d0[j0]; dc[cc[b,(k-2)*64+s]] = comp[s].
    """
    R, N = pidx.shape
    flat = pidx.astype(np.int64)
    keys = (flat + NPIX * np.arange(R, dtype=np.int64)[:, None]).ravel()
    order = np.argsort(keys, kind="stable")
    sk = keys[order]
    pos = np.arange(R * N)
    first = np.r_[True, sk[1:] != sk[:-1]]
    grp_start = np.maximum.accumulate(np.where(first, pos, 0))
    rank = pos - grp_start
    row = order // N
    j = (order % N).astype(np.int16)
    firstj = j[grp_start]
    n_ranks = int(rank.max()) + 1

    inv = np.full((R, NPIX), -1, np.int16)
    m0 = rank == 0
    inv[row[m0], flat.ravel()[order[m0]]] = j[m0]

    c1 = np.full((R, N), -1, np.int16)
    m1 = rank == 1
    c1[row[m1], firstj[m1]] = j[m1]

    # compaction slots for pixels used >= 3 times (their rank-2 entry)
    pc = np.full((R, N), -1, np.int16)
    NCC = max(n_ranks - 2, 1) * NCOMP
    cc = np.full((R, NCC), -1, np.int16)
    m2 = rank == 2
    rows2 = row[m2]
    rfirst = np.r_[True, rows2[1:] != rows2[:-1]] if rows2.size else np.array([], bool)
    rstart = (
        np.maximum.accumulate(np.where(rfirst, np.arange(rows2.size), 0))
        if rows2.size
        else np.array([], np.int64)
    )
    slot = (np.arange(rows2.size) - rstart).astype(np.int16)
    assert slot.size == 0 or slot.max() < NCOMP, "NCOMP overflow"
    pc[rows2, firstj[m2]] = slot
    # slot lookup per group for ranks >= 2
    slot_of_group = {}
    g2 = grp_start[m2]
    for gg, ss, rr in zip(g2, slot, rows2):
        slot_of_group[gg] = ss
    for k in range(2, n_ranks):
        mk = rank == k
        gk = grp_start[mk]
        sk_ = np.array([slot_of_group[g] for g in gk], dtype=np.int64)
        cc[row[mk], (k - 2) * NCOMP + sk_] = j[mk]
    return inv, c1, pc, cc, n_ranks


def _prep_inputs(x, theta, pair_idx):
    """Full inputs -> list of per-core input maps (host-side sharding and
    index re-encoding only; model data x/theta untouched beyond reshape)."""
    x = np.ascontiguousarray(np.asarray(x, dtype=np.float32).reshape(B_FULL, NPIX))
    theta = np.ascontiguousarray(np.asarray(theta, dtype=np.float32).reshape(1, NCLS))
    pidx = np.asarray(pair_idx).reshape(B_FULL, NIDX)
    inv, c1, pc, cc, n_ranks = _index_prep(pidx)
    in_maps = []
    for k in range(N_CORES):
        sl = slice(k * B, (k + 1) * B)
        in_maps.append(
            {
                "x": x[sl],
                "inv": inv[sl],
                "c1": c1[sl],
                "pc": pc[sl],
                "cc": cc[sl],
                "theta": theta,
            }
        )
    return in_maps, n_ranks


_CACHED = {}


def kernel(x, theta, pair_idx):
    from concourse.bass_utils import run_bass_kernel_spmd

    in_maps, n_ranks = _prep_inputs(x, theta, pair_idx)
    n_ranks = max(n_ranks, 3)
    if ("nc", n_ranks) not in _CACHED:
        _CACHED[("nc", n_ranks)] = build_kernel(n_ranks=n_ranks)
    nc = _CACHED[("nc", n_ranks)]
    res = run_bass_kernel_spmd(nc, in_maps, core_ids=list(range(N_CORES)))
    out = np.concatenate([r["out"] for r in res.results], axis=0)
    return out.reshape(B_FULL, NIDX, NCLS)
